# revision 26
# baseline (speedup 1.0000x reference)
"""Trainium2 Bass kernel for nn_AttentionalGNN (6-layer self/cross GNN + Sinkhorn).

Sharding: desc1's 4096 tokens are split into 8 contiguous blocks of 512 (one
per core).  Small tensors (desc0/desc2/desc3) and all weights are replicated;
heavy ops (desc1 self-attention, pair-MLP cross attention, desc1-side MLPs)
are computed on the owned block only.  One AllGather per layer rebuilds the
full desc1 (plus, on cross layers, partial-softmax partials for the ops whose
keys are sharded).  Softmaxes run unstabilized (score ranges verified small)
in a transposed [key-part, query-free] layout so no transposes are needed in
the hot loop.
"""

import math
import numpy as np

import concourse.bass as bass
import concourse.bacc as bacc
import concourse.mybir as mybir
import concourse.tile as tile
from concourse import bass_utils

F32 = mybir.dt.float32
AF = mybir.ActivationFunctionType
ALU = mybir.AluOpType

NCORES = 8
D = 128
N0, N1, N2, N3 = 8, 4096, 256, 256
H, DH = 4, 32
NS = N1 // NCORES  # 512
LAYER_NAMES = ["self", "cross", "self", "cross", "self", "cross"]
ISQ = 1.0 / math.sqrt(DH)
NORM = -math.log(N1 + N0)  # -log(m+n), m=4096 n=8
OT_ITERS = 5
DEBUG = False

# head-channel permutation: ours c' = h*32+d  <-  reference c = d*4+h
PERM = np.array([(c % DH) * H + c // DH for c in range(D)])

# cross-layer payload row map
PR_D1 = 0        # rows 0:128   desc1 block (cols 0:512)
PR_M21 = 128     # rows 128:256 cols 0:256 msg21 partial; cols 256:264 msg0 partial; row 128 cols 272:280 l0 partial
PR_L21 = 256     # rows 256:384 l21 partial (rows 32h used, cols 0:256)
PR_S1 = 384      # rows 384:512 score1 raw [128, 32] (layer 5 only)
R_CROSS = 384
R_CROSS5 = 512
R_SELF = 128


def _np(x):
    return np.asarray(x, dtype=np.float32)


def _pack_mha(p):
    (wq, bq), (wk, bk), (wv, bv) = [(_np(w), _np(b)) for w, b in p["proj"]]
    wm, bm = _np(p["merge"][0]), _np(p["merge"][1])
    out = {
        "wqT": wq[PERM, :].T.copy(),
        "bq": bq[PERM].reshape(D, 1).copy(),
        "wkT": wk[PERM, :].T.copy(),
        "bk": bk[PERM].reshape(D, 1).copy(),
        "wvT": wv[PERM, :].T.copy(),
        "wmT": wm[:, PERM].T.copy(),
        "bm": (wm @ bv + bm).reshape(D, 1).copy(),
    }
    return out


def _pack_mlp(p):
    (w1, b1), (w2, b2) = [(_np(w), _np(b)) for w, b in p]
    w1T = w1.T  # (2D, 2D)
    w2T = w2.T  # (2D, D)
    return {
        "w1Ta": w1T[0:D, :].copy(),
        "w1Tb": w1T[D : 2 * D, :].copy(),
        "b1a": b1[0:D].reshape(D, 1).copy(),
        "b1b": b1[D : 2 * D].reshape(D, 1).copy(),
        "w2Ta": w2T[0:D, :].copy(),
        "w2Tb": w2T[D : 2 * D, :].copy(),
        "b2": b2.reshape(D, 1).copy(),
    }


def _pack_xattn(p):
    (w1, b1), (w2, b2) = [(_np(w), _np(b)) for w, b in p["mlp"]]
    w1T = w1.T  # (3D, D)
    return {
        "w1qT": w1T[0:D, :].copy(),
        "w1kT": w1T[D : 2 * D, :].copy(),
        "w1dT": w1T[2 * D : 3 * D, :].copy(),
        "b1": b1.reshape(D, 1).copy(),
        "w2T": w2.T.copy(),  # (D, 1)
    }


def pack_params(params):
    flat = {}
    for i, name in enumerate(LAYER_NAMES):
        ap, php, ghp = params["attn"][i], params["phattn"][i], params["ghattn"][i]
        if name == "cross":
            for k, v in _pack_xattn(ap["attn"]).items():
                flat[f"L{i}_a_{k}"] = v
        else:
            for k, v in _pack_mha(ap["attn"]).items():
                flat[f"L{i}_a_{k}"] = v
        for k, v in _pack_mlp(ap["mlp"]).items():
            flat[f"L{i}_am_{k}"] = v
        for k, v in _pack_mha(php["attn"]).items():
            flat[f"L{i}_p_{k}"] = v
        for k, v in _pack_mlp(php["mlp"]).items():
            flat[f"L{i}_pm_{k}"] = v
        for k, v in _pack_mha(ghp["attn"]).items():
            flat[f"L{i}_g_{k}"] = v
        for k, v in _pack_mlp(ghp["mlp"]).items():
            flat[f"L{i}_gm_{k}"] = v
    return flat


def fview(ap, free_dims, offset_elems=0):
    """Replace the free dims of a [P, ...] AP (keeps the partition dim)."""
    return bass.AP(ap.tensor, ap.offset + offset_elems, [list(ap.ap[0])] + [list(d) for d in free_dims])


class Kern:
    def __init__(self, nc, tc):
        self.nc = nc
        self.tc = tc
        self.w = tc.alloc_tile_pool(name="w", bufs=2)
        self.sb = tc.alloc_tile_pool(name="sb", bufs=2)
        self.sb1 = tc.alloc_tile_pool(name="sb1", bufs=1)
        self.pst = tc.alloc_tile_pool(name="pst", bufs=1)  # persistent state
        self.ps = tc.alloc_tile_pool(name="ps", bufs=1, space="PSUM")
        self.dram = tc.alloc_tile_pool(name="dram", bufs=1, space="DRAM")
        self.wt = {}

    # ---------- weights ----------
    def layer_weights(self, li, wspec_layer, tensors):
        wt = {}
        for name, shape in wspec_layer:
            field = name.split("_", 1)[1]  # strip L{i}_ prefix
            t = self.w.tile(list(shape), F32, tag=f"w_{field}")
            self.nc.sync.dma_start(t[:], tensors[name][:])
            wt[field] = t
        return wt

    # ---------- primitives ----------
    def mm(self, psum_ap, lhsT, rhs, start=True, stop=True, tp=None):
        self.nc.tensor.matmul(psum_ap, lhsT, rhs, start=start, stop=stop,
                              tile_position=tp, skip_group_check=True)

    def evac(self, out_ap, psum_ap, bias=None, relu=False):
        v = self.nc.vector
        if bias is not None and relu:
            v.tensor_scalar(out_ap, psum_ap, bias, 0.0, ALU.add, ALU.max)
        elif bias is not None:
            v.tensor_scalar_add(out_ap, psum_ap, bias)
        elif relu:
            v.tensor_scalar_max(out_ap, psum_ap, 0.0)
        else:
            v.tensor_copy(out_ap, psum_ap)

    def bcast_rows(self, ps_out, row_ap, base, nrows):
        """row at partition `base` -> ps_out[base:base+nrows, :] via PE outer product."""
        ones = self.ones
        n = row_ap.ap[-1][1]
        self.mm(ps_out, ones[base : base + 1, 0:nrows], row_ap, tp=((base // 32) * 32, (base // 32) * 32))

    # ---------- MHA on a query block ----------
    def mha(self, tag, xq, nq, kv, nk, W, partial=False):
        nc, sb, ps = self.nc, self.sb, self.ps
        nkt = (nk + 127) // 128

        psq = ps.tile([128, 512], F32, tag="psA")
        self.mm(psq[:, 0:nq], W["wqT"][:], xq)
        qh = sb.tile([128, 512], mybir.dt.bfloat16, tag="qh")
        self.evac(qh[:, 0:nq], psq[:, 0:nq], bias=W["bq"][:, 0:1])

        ps_msg = ps.tile([128, 512], F32, tag="psMSG")
        ps_l = ps.tile([128, 512], F32, tag="psL")
        esum = []
        for h in range(H):
            t_esum = self.sb1.tile([128, nq], F32, tag=f"esum{h}")
            esum.append(t_esum)

        for kt in range(nkt):
            kk = min(128, nk - kt * 128)
            ksl = slice(kt * 128, kt * 128 + kk)
            # kh tile [128, kk]
            ps_kh = ps.tile([128, 512], F32, tag="psB")
            self.mm(ps_kh[:, 0:kk], W["wkT"][:], kv[:, ksl])
            kh = sb.tile([128, 128], mybir.dt.bfloat16, tag="kh")
            self.evac(kh[:, 0:kk], ps_kh[:, 0:kk], bias=W["bk"][:, 0:1])
            # vT tile [kk, 128]
            ps_vT = ps.tile([128, 512], F32, tag="psL")
            self.mm(ps_vT[0:kk, 0:128], kv[:, ksl], W["wvT"][:])
            vT = sb.tile([128, 128], mybir.dt.bfloat16, tag="vT")
            self.evac(vT[0:kk, :], ps_vT[0:kk, 0:128])
            first, last = kt == 0, kt == nkt - 1
            for h in range(H):
                hs = slice(32 * h, 32 * h + 32)
                ps_s = ps.tile([128, 512], F32, tag=f"psS{h}")
                self.mm(ps_s[0:kk, 0:nq], kh[hs, 0:kk], qh[hs, 0:nq], tp=(32 * h, 0))
                es = sb.tile([128, nq], mybir.dt.bfloat16, tag=f"es{h}")
                nc.scalar.activation(es[0:kk, 0:nq], ps_s[0:kk, 0:nq], AF.Exp, scale=ISQ)
                self.mm(ps_msg[hs, 0:nq], vT[0:kk, hs], es[0:kk, 0:nq],
                        start=first, stop=last, tp=(0, 32 * h))
                if first:
                    nc.vector.tensor_copy(esum[h][0:kk, :], es[0:kk, 0:nq])
                else:
                    nc.vector.tensor_tensor(esum[h][:], esum[h][:], es[:, 0:nq], ALU.add)
        kfin = min(128, nk)
        for h in range(H):
            self.mm(ps_l[32 * h : 32 * h + 1, 0:nq], self.ones[0:kfin, 0:1],
                    esum[h][0:kfin, :], start=True, stop=True, tp=(0, 32 * h))

        if partial:
            msg_sb = sb.tile([128, nq], F32, tag="pmsg")
            l_sb = sb.tile([128, nq], F32, tag="pl")
            self.evac(msg_sb[:], ps_msg[:, 0:nq])
            self.evac(l_sb[:], ps_l[:, 0:nq])
            return msg_sb, l_sb

        msgn = self.normalize_msg(ps_msg, ps_l, nq)
        # merge conv
        ps_m = ps.tile([128, 512], F32, tag="psA")
        self.mm(ps_m[:, 0:nq], W["wmT"][:], msgn[:, 0:nq])
        attn = sb.tile([128, nq], F32, tag=tag)
        self.evac(attn[:], ps_m[:, 0:nq], bias=W["bm"][:, 0:1])
        return attn

    def normalize_msg(self, ps_msg, ps_l, nq, msg_sb=None, l_sb=None):
        """msg/l either in PSUM (ps_msg/ps_l) or SBUF; returns normalized msg [128, nq] SBUF."""
        nc, sb, ps = self.nc, self.sb, self.ps
        src_l = ps_l if l_sb is None else l_sb
        recip = sb.tile([128, nq], F32, tag="recip")
        for h in range(H):
            r = slice(32 * h, 32 * h + 1)
            nc.vector.reciprocal(recip[r, 0:nq], src_l[r, 0:nq])
        ps_bc = ps.tile([128, 512], F32, tag="psB")
        for h in range(H):
            self.bcast_rows(ps_bc[32 * h : 32 * h + 32, 0:nq], recip[32 * h : 32 * h + 1, 0:nq], 32 * h, 32)
        msgn = sb.tile([128, nq], F32, tag="msgn")
        if msg_sb is None:
            nc.vector.tensor_copy(msgn[:], ps_msg[:, 0:nq])
            nc.vector.tensor_tensor(msgn[:], msgn[:], ps_bc[:, 0:nq], ALU.mult)
        else:
            nc.vector.tensor_tensor(msgn[:], msg_sb[:, 0:nq], ps_bc[:, 0:nq], ALU.mult)
        return msgn

    # ---------- two-layer prop MLP on [x; attn] ----------
    def prop_mlp(self, tag, x, attn, M, nq, out_ap=None, out_bias_add=None):
        nc, sb, ps = self.nc, self.sb, self.ps
        hs = []
        for m in range(2):
            msl = slice(128 * m, 128 * m + 128)
            ph = ps.tile([128, 512], F32, tag="psA")
            self.mm(ph[:, 0:nq], M["w1Ta"][:, msl], x, start=True, stop=False)
            self.mm(ph[:, 0:nq], M["w1Tb"][:, msl], attn, start=False, stop=True)
            hm = sb.tile([128, nq], F32, tag=f"h{m}")
            self.evac(hm[:], ph[:, 0:nq], bias=M["b1a" if m == 0 else "b1b"][:, 0:1], relu=True)
            hs.append(hm)
        pd = ps.tile([128, 512], F32, tag="psB")
        self.mm(pd[:, 0:nq], M["w2Ta"][:], hs[0][:], start=True, stop=False)
        self.mm(pd[:, 0:nq], M["w2Tb"][:], hs[1][:], start=False, stop=True)
        if out_ap is None:
            delta = sb.tile([128, nq], F32, tag=tag)
            self.evac(delta[:], pd[:, 0:nq], bias=M["b2"][:, 0:1])
            return delta
        else:
            self.evac(out_ap, pd[:, 0:nq], bias=M["b2"][:, 0:1])
            return None


def build(nc, DBG):
    di = {}

    def inp(name, shape):
        di[name] = nc.dram_tensor(name, list(shape), F32, kind="ExternalInput").ap()
        return di[name]

    inp("desc0", (D, N0)); inp("desc1", (D, N1)); inp("desc2", (D, N2)); inp("desc3", (D, N3))
    inp("desc1_loc", (D, NS)); inp("dist0_loc", (D, 8 * NS)); inp("dist1_loc", (D, 8 * NS))
    inp("maskf", (1, N0)); inp("alpha_cols", (D, 32)); inp("alpha_row9", (1, 9)); inp("ident", (D, D))
    out_t = nc.dram_tensor("out", [1, N0], F32, kind="ExternalOutput").ap()
    dbg = {}
    if DBG:
        for nm, shape in [("dbg_desc0", (D, N0)), ("dbg_desc2", (D, N2)), ("dbg_desc3", (D, N3)),
                          ("dbg_d1loc", (D, NS)), ("dbg_z", (D, 256)), ("dbg_row0", (1, 16))]:
            dbg[nm] = nc.dram_tensor(nm, list(shape), F32, kind="ExternalOutput").ap()

    wspec = []
    wspec_by_layer = []
    for i, name in enumerate(LAYER_NAMES):
        w0 = len(wspec)
        if name == "cross":
            wspec += [(f"L{i}_a_w1qT", (D, D)), (f"L{i}_a_w1kT", (D, D)), (f"L{i}_a_w1dT", (D, D)),
                      (f"L{i}_a_b1", (D, 1)), (f"L{i}_a_w2T", (D, 1))]
        else:
            wspec += [(f"L{i}_a_{k}", (D, D)) for k in ["wqT", "wkT", "wvT", "wmT"]]
            wspec += [(f"L{i}_a_{k}", (D, 1)) for k in ["bq", "bk", "bm"]]
        for g in ["am", "pm", "gm"]:
            wspec += [(f"L{i}_{g}_w1Ta", (D, 2 * D)), (f"L{i}_{g}_w1Tb", (D, 2 * D)),
                      (f"L{i}_{g}_w2Ta", (D, D)), (f"L{i}_{g}_w2Tb", (D, D))]
            wspec += [(f"L{i}_{g}_b1a", (D, 1)), (f"L{i}_{g}_b1b", (D, 1)), (f"L{i}_{g}_b2", (D, 1))]
        for g in ["p", "g"]:
            wspec += [(f"L{i}_{g}_{k}", (D, D)) for k in ["wqT", "wkT", "wvT", "wmT"]]
            wspec += [(f"L{i}_{g}_{k}", (D, 1)) for k in ["bq", "bk", "bm"]]
        wspec_by_layer.append(wspec[w0:])
    for n, s in wspec:
        di[n] = nc.dram_tensor(n, list(s), F32, kind="ExternalInput").ap()

    with tile.TileContext(nc) as tc:
        K = Kern(nc, tc)
        sb, ps, pst, dram = K.sb, K.ps, K.pst, K.dram

        ones = K.w.tile([128, 128], F32, tag="ones")
        nc.gpsimd.memset(ones[:], 1.0)
        K.ones = ones
        ident = K.w.tile([128, 128], F32, tag="ident")
        nc.sync.dma_start(ident[:], di["ident"][:])
        maskf = K.w.tile([1, N0], F32, tag="maskf")
        nc.sync.dma_start(maskf[:], di["maskf"][:])


        # persistent state
        d0 = pst.tile([D, N0], F32, tag="d0")
        d2 = pst.tile([D, N2], F32, tag="d2")
        d3 = pst.tile([D, N3], F32, tag="d3")
        d1f = pst.tile([D, N1], F32, tag="d1f")
        d1l = pst.tile([D, NS], F32, tag="d1l")
        for t, n in [(d0, "desc0"), (d2, "desc2"), (d3, "desc3"), (d1f, "desc1"),
                     (d1l, "desc1_loc")]:
            nc.sync.dma_start(t[:], di[n][:])

        z_tile = [None]  # score1 container post layer 5

        LW = [None]

        def mw(i, g, k):
            return LW[0][f"{g}_{k}"]

        def Wd(i, g):
            return {k: mw(i, g, k) for k in ["wqT", "bq", "wkT", "bk", "wvT", "wmT", "bm"]}

        def Md(i, g):
            return {k: mw(i, g, k) for k in ["w1Ta", "w1Tb", "b1a", "b1b", "w2Ta", "w2Tb", "b2"]}

        def do_gather(li, payload_rows, writes):
            pay = dram.tile([payload_rows, 512], F32, tag=f"pay{li}")
            for (r, c, srcap) in writes:
                p = srcap.ap[0][1]
                nc.sync.dma_start(pay[r : r + p, c : c + srcap.free_size()], srcap)
            gath = dram.tile([payload_rows * 8, 512], F32, tag=f"gath{li}", addr_space="Shared")
            nc.gpsimd.collective_compute(
                "AllGather", ALU.bypass, replica_groups=[list(range(NCORES))],
                ins=[pay.opt()], outs=[gath.opt()])
            return gath

        def reload_d1f(gath, R):
            # gathered [R*8, 512]: core s rows [R*s : R*s+128] -> d1f[:, 512s:...]
            gap = gath[:]
            src = bass.AP(gap.tensor, gap.offset, [[512, 128], [R * 512, 8], [1, 512]])
            nc.sync.dma_start(fview(d1f[:], [[512, 8], [1, 512]]), src)

        # ============ layers ============
        for li, lname in enumerate(LAYER_NAMES):
            LW[0] = K.layer_weights(li, wspec_by_layer[li], di)
            if lname == "self":
                a1 = K.mha("attn", d1l[:], NS, d1f[:], N1, Wd(li, "a"))
                dl1 = K.prop_mlp("delta", d1l[:], a1[:], Md(li, "am"), NS)
                nc.vector.tensor_add(d1l[:], d1l[:], dl1[:])
                gath = do_gather(li, R_SELF, [(PR_D1, 0, d1l[:])])
                reload_d1f(gath, R_SELF)
                a0 = K.mha("attn", d0[:], N0, d0[:], N0, Wd(li, "a"))
                dl0 = K.prop_mlp("delta", d0[:], a0[:], Md(li, "am"), N0)
                nc.vector.tensor_add(d0[:], d0[:], dl0[:])
                a2 = K.mha("attn", d2[:], N2, d2[:], N2, Wd(li, "p"))
                dl2 = K.prop_mlp("delta", d2[:], a2[:], Md(li, "pm"), N2)
                nc.vector.tensor_add(d2[:], d2[:], dl2[:])
                a3 = K.mha("attn", d3[:], N3, d3[:], N3, Wd(li, "g"))
                dl3 = K.prop_mlp("delta", d3[:], a3[:], Md(li, "gm"), N3)
                nc.vector.tensor_add(d3[:], d3[:], dl3[:])
            else:
                # ---- delta0 partials (pair-MLP, queries=desc0, keys=my desc1 block) ----
                # transpose my desc1 block -> d1lT [i1l, c] (4 tiles of 128)
                d1lT = sb.tile([128, 512], F32, tag="d1lT")
                for t in range(4):
                    ps_t = ps.tile([128, 512], F32, tag="psB")
                    nc.tensor.transpose(ps_t[:, 0:128], d1l[:, 128 * t : 128 * t + 128], ident[:])
                    K.evac(d1lT[:, 128 * t : 128 * t + 128], ps_t[:, 0:128])
                es0 = sb.tile([N0, 512], F32, tag="es0")
                ps_sk = ps.tile([128, 512], F32, tag="psB")
                K.mm(ps_sk[:, 0:NS], mw(li, "a", "w1kT")[:], d1l[:])
                Sk = K.sb1.tile([128, NS], F32, tag="Sk")
                K.evac(Sk[:], ps_sk[:, 0:NS])
                ps_qb = ps.tile([128, 512], F32, tag="psB")
                K.mm(ps_qb[:, 0:N0], mw(li, "a", "w1qT")[:], d0[:])
                Qbb = sb.tile([128, N0], F32, tag="Qbb")
                K.evac(Qbb[:], ps_qb[:, 0:N0], bias=mw(li, "a", "b1")[:, 0:1])
                for i0 in range(N0):
                    dch0 = sb.tile([128, NS], F32, tag="dch")
                    nc.sync.dma_start(dch0[:], di["dist0_loc"][:, NS * i0 : NS * i0 + NS])
                    ph = ps.tile([128, 512], F32, tag="psA")
                    K.mm(ph[:, 0:NS], mw(li, "a", "w1dT")[:], dch0[:])
                    nc.vector.tensor_tensor(ph[:, 0:NS], ph[:, 0:NS], Sk[:], ALU.add)
                    hid = sb.tile([128, NS], F32, tag="hid0")
                    nc.scalar.activation(hid[:], ph[:, 0:NS], AF.Relu, bias=Qbb[:, i0 : i0 + 1])
                    ps_s = ps.tile([128, 512], F32, tag="psL")
                    K.mm(ps_s[0:1, 0:NS], mw(li, "a", "w2T")[:], hid[:])
                    er = sb.tile([1, NS], F32, tag="er0")
                    nc.scalar.activation(er[:], ps_s[0:1, 0:NS], AF.Exp)
                    nc.sync.dma_start(es0[i0 : i0 + 1, :], er[:])
                # transpose es0 [8, 512] -> es0T tiles [128, 8] x4 ; partial msg0/l0
                ps_m0 = ps.tile([128, 512], F32, tag="psS0")
                ps_l0 = ps.tile([128, 512], F32, tag="psS1")
                for t in range(4):
                    ps_t = ps.tile([128, 512], F32, tag="psB")
                    nc.tensor.transpose(ps_t[0:128, 0:N0], es0[0:N0, 128 * t : 128 * t + 128], ident[0:N0, 0:N0])
                    e0T = sb.tile([128, N0], F32, tag="e0T")
                    K.evac(e0T[:], ps_t[0:128, 0:N0])
                    K.mm(ps_m0[:, 0:N0], d1lT[:, 128 * t : 128 * t + 128], e0T[:],
                         start=(t == 0), stop=(t == 3))
                    K.mm(ps_l0[0:1, 0:N0], ones[:, 0:1], e0T[:], start=(t == 0), stop=(t == 3))
                m0p = sb.tile([128, N0], F32, tag="m0p")
                l0p = sb.tile([1, N0], F32, tag="l0p")
                K.evac(m0p[:], ps_m0[:, 0:N0])
                K.evac(l0p[:], ps_l0[0:1, 0:N0])

                # ---- delta1 (pair-MLP, queries=my block, keys=desc0) ----
                sc1 = K.sb1.tile([1, 8 * NS], F32, tag="sc1")
                ps_q1 = ps.tile([128, 512], F32, tag="psB")
                K.mm(ps_q1[:, 0:NS], mw(li, "a", "w1qT")[:], d1l[:])
                Qloc = K.sb1.tile([128, NS], F32, tag="Qloc")
                K.evac(Qloc[:], ps_q1[:, 0:NS])
                ps_k8 = ps.tile([128, 512], F32, tag="psB")
                K.mm(ps_k8[:, 0:N0], mw(li, "a", "w1kT")[:], d0[:])
                Kt8 = sb.tile([128, N0], F32, tag="Kt8")
                K.evac(Kt8[:], ps_k8[:, 0:N0])
                for c8 in range(8):
                    csl = slice(NS * c8, NS * c8 + NS)
                    dch1 = sb.tile([128, NS], F32, tag="dch")
                    nc.sync.dma_start(dch1[:], di["dist1_loc"][:, csl])
                    ph = ps.tile([128, 512], F32, tag="psA")
                    K.mm(ph[:, 0:NS], mw(li, "a", "w1dT")[:], dch1[:])
                    nc.vector.tensor_tensor(ph[:, 0:NS], ph[:, 0:NS],
                                            fview(Qloc[:], [[1, 64], [0, 8]], offset_elems=64 * c8), ALU.add)
                    nc.vector.tensor_tensor(ph[:, 0:NS], ph[:, 0:NS],
                                            fview(Kt8[:], [[0, 64], [1, 8]]), ALU.add)
                    hid = sb.tile([128, NS], F32, tag="hid1")
                    nc.scalar.activation(hid[:], ph[:, 0:NS], AF.Relu, bias=mw(li, "a", "b1")[:, 0:1])
                    ps_s = ps.tile([128, 512], F32, tag="psL")
                    K.mm(ps_s[0:1, 0:NS], mw(li, "a", "w2T")[:], hid[:])
                    nc.vector.tensor_copy(sc1[0:1, csl], ps_s[0:1, 0:NS])
                # reshape to S1t [128, (4 m, 8 ik)]  (iq_local = 4p + m)
                S1t = sb.tile([128, 32], F32, tag="S1t", )
                s1ap = sc1[:]
                src = bass.AP(s1ap.tensor, s1ap.offset, [[1, 1], [32, 128], [8, 4], [1, 8]])
                nc.sync.dma_start(fview(S1t[:], [[8, 4], [1, 8]]), src)
                eS1 = sb.tile([128, 32], F32, tag="eS1")
                nc.scalar.activation(eS1[:], S1t[:], AF.Exp)
                # transpose -> E1T [32, 128]; shift m-groups to partition 0
                ps_t = ps.tile([128, 512], F32, tag="psB")
                nc.tensor.transpose(ps_t[0:32, 0:128], eS1[:], ident[:])
                E1T = sb.tile([32, 128], F32, tag="E1T")
                K.evac(E1T[:], ps_t[0:32, 0:128])
                E1m = sb.tile([8, 4 * 128], F32, tag="E1m")
                for m in range(4):
                    nc.sync.dma_start(E1m[0:8, 128 * m : 128 * m + 128], E1T[8 * m : 8 * m + 8, :])
                # desc0^T
                ps_t0 = ps.tile([128, 512], F32, tag="psB")
                nc.tensor.transpose(ps_t0[0:N0, 0:128], d0[:], ident[:])
                d0T = sb.tile([N0, 128], F32, tag="d0T")
                K.evac(d0T[:], ps_t0[0:N0, 0:128])
                delta1 = sb.tile([128, NS], F32, tag="delta1")
                for m in range(4):
                    ps_m1 = ps.tile([128, 512], F32, tag="psS0")
                    K.mm(ps_m1[:, 0:128], d0T[:], E1m[0:8, 128 * m : 128 * m + 128])
                    ps_l1 = ps.tile([128, 512], F32, tag="psS1")
                    K.mm(ps_l1[0:1, 0:128], ones[0:8, 0:1], E1m[0:8, 128 * m : 128 * m + 128])
                    r1 = sb.tile([1, 128], F32, tag="r1")
                    nc.vector.reciprocal(r1[:], ps_l1[0:1, 0:128])
                    ps_bc = ps.tile([128, 512], F32, tag="psMSG")
                    K.mm(ps_bc[:, 0:128], ones[0:1, 0:128], r1[:], tp=(0, 0))
                    m1n = sb.tile([128, 128], F32, tag="m1n")
                    nc.vector.tensor_copy(m1n[:], ps_m1[:, 0:128])
                    nc.vector.tensor_tensor(m1n[:], m1n[:], ps_bc[:, 0:128], ALU.mult)
                    xv = fview(d1l[:], [[4, 128]], offset_elems=m)
                    ov = fview(delta1[:], [[4, 128]], offset_elems=m)
                    K.prop_mlp("pd1", xv, m1n[:], Md(li, "am"), 128, out_ap=ov)

                # ---- delta12 / delta21 / ghp ----
                a12 = K.mha("attn", d1l[:], NS, d2[:], N2, Wd(li, "p"))
                dl12 = K.prop_mlp("dl12", d1l[:], a12[:], Md(li, "pm"), NS)
                m21, l21 = K.mha("a21", d2[:], N2, d1l[:], NS, Wd(li, "p"), partial=True)
                # ---- updates (desc1 block) after all readers of old d1l ----
                nc.vector.tensor_add(d1l[:], d1l[:], delta1[:])
                nc.vector.tensor_add(d1l[:], d1l[:], dl12[:])

                # ---- gather (overlaps with ghp props below) ----
                R = R_CROSS5 if li == 5 else R_CROSS
                writes = [(PR_D1, 0, d1l[:]), (PR_M21, 0, m21[:]), (PR_M21, 256, m0p[:]),
                          (PR_L21, 0, l21[:])]
                if li == 5:
                    writes.append((PR_S1, 0, S1t[:]))
                pay = dram.tile([R, 512], F32, tag=f"pay{li}")
                for (r, c, srcap) in writes:
                    p = srcap.ap[0][1]
                    nc.sync.dma_start(pay[r : r + p, c : c + srcap.free_size()], srcap)
                nc.sync.dma_start(pay[PR_M21 : PR_M21 + 1, 272:280], l0p[:])
                gath = dram.tile([R * 8, 512], F32, tag=f"gath{li}", addr_space="Shared")
                nc.gpsimd.collective_compute(
                    "AllGather", ALU.bypass, replica_groups=[list(range(NCORES))],
                    ins=[pay.opt()], outs=[gath.opt()])
                reload_d1f(gath, R)

                a03 = K.mha("attn", d0[:], N0, d3[:], N3, Wd(li, "g"))
                dl03 = K.prop_mlp("dl03", d0[:], a03[:], Md(li, "gm"), N0)
                a30 = K.mha("attn", d3[:], N3, d0[:], N0, Wd(li, "g"))
                dl30 = K.prop_mlp("delta", d3[:], a30[:], Md(li, "gm"), N3)
                nc.vector.tensor_add(d3[:], d3[:], dl30[:])

                # ---- post-gather: sum partials ----
                gap = gath[:]
                acc1 = sb.tile([128, 512], F32, tag="acc1")
                acc2 = sb.tile([128, 256], F32, tag="acc2")
                for s in range(8):
                    t1 = sb.tile([128, 512], F32, tag="gtmp1")
                    src = bass.AP(gap.tensor, gap.offset + (R * s + PR_M21) * 512, [[512, 128], [1, 512]])
                    nc.sync.dma_start(t1[:], src)
                    if s == 0:
                        nc.vector.tensor_copy(acc1[:], t1[:])
                    else:
                        nc.vector.tensor_add(acc1[:], acc1[:], t1[:])
                    t2 = sb.tile([128, 256], F32, tag="gtmp2")
                    src2 = bass.AP(gap.tensor, gap.offset + (R * s + PR_L21) * 512, [[512, 128], [1, 256]])
                    nc.sync.dma_start(t2[:], src2)
                    if s == 0:
                        nc.vector.tensor_copy(acc2[:], t2[:])
                    else:
                        nc.vector.tensor_add(acc2[:], acc2[:], t2[:])
                # delta21: normalize msg21 (acc1 cols 0:256, l in acc2) + merge + mlp
                m21n = K.normalize_msg(None, None, N2, msg_sb=acc1, l_sb=acc2)
                ps_m = ps.tile([128, 512], F32, tag="psA")
                K.mm(ps_m[:, 0:N2], mw(li, "p", "wmT")[:], m21n[:, 0:N2])
                a21 = sb.tile([128, N2], F32, tag="a21f")
                K.evac(a21[:], ps_m[:, 0:N2], bias=mw(li, "p", "bm")[:, 0:1])
                dl21 = K.prop_mlp("delta", d2[:], a21[:], Md(li, "pm"), N2)
                # delta0: normalize msg0 (acc1 cols 256:264 / l row at [0:1, 272:280])
                r0 = sb.tile([1, N0], F32, tag="r0")
                nc.vector.reciprocal(r0[:], acc1[0:1, 272:280])
                ps_bc0 = ps.tile([128, 512], F32, tag="psB")
                K.mm(ps_bc0[:, 0:N0], ones[0:1, 0:128], r0[:], tp=(0, 0))
                m0n = sb.tile([128, N0], F32, tag="m0n")
                nc.vector.tensor_tensor(m0n[:], acc1[:, 256:264], ps_bc0[:, 0:N0], ALU.mult)
                nc.vector.tensor_add(d2[:], d2[:], dl21[:])
                dl0 = K.prop_mlp("delta", d0[:], m0n[:], Md(li, "am"), N0)
                nc.vector.tensor_add(d0[:], d0[:], dl0[:])
                nc.vector.tensor_add(d0[:], d0[:], dl03[:])

                if li == 5:
                    # Z [128, (8 s, 4 m, 8 ik)] raw score1
                    zt = pst.tile([128, 256], F32, tag="Z")
                    src = bass.AP(gap.tensor, gap.offset + PR_S1 * 512,
                                  [[512, 128], [R * 512, 8], [8, 4], [1, 8]])
                    nc.sync.dma_start(fview(zt[:], [[32, 8], [8, 4], [1, 8]]), src)
                    z_tile[0] = zt

        # ============ Sinkhorn + output ============
        zt = z_tile[0]
        # log_softmax over rows (per ik): E=exp(Z); colsum via ones-matmul; LSE=ln(32*mean)... use sum directly
        E = sb.tile([128, 256], F32, tag="E")
        nc.scalar.activation(E[:], zt[:], AF.Exp)
        ps_cs = ps.tile([128, 512], F32, tag="psA")
        K.mm(ps_cs[0:1, 0:256], ones[:, 0:1], E[:])
        csr = sb.tile([1, 256], F32, tag="csr")
        nc.vector.tensor_copy(csr[:], ps_cs[0:1, 0:256])
        sum8 = sb.tile([1, 8], F32, tag="sum8")
        nc.vector.tensor_reduce(sum8[:], fview(csr[:], [[1, 8], [8, 32]]), mybir.AxisListType.X, ALU.add)
        lse8 = sb.tile([1, 8], F32, tag="lse8")
        nc.scalar.activation(lse8[:], sum8[:], AF.Ln)
        ps_lb = ps.tile([128, 512], F32, tag="psB")
        K.mm(ps_lb[:, 0:8], ones[0:1, 0:128], lse8[:], tp=(0, 0))
        lseb = sb.tile([128, 8], F32, tag="lseb")
        nc.vector.tensor_copy(lseb[:], ps_lb[:, 0:8])
        # couplings C [128, (32 t, 9)]
        C = pst.tile([128, 288], F32, tag="C")
        zls_v = fview(C[:], [[9, 32], [1, 8]])
        nc.vector.tensor_tensor(zls_v, zt[:], fview(lseb[:], [[0, 32], [1, 8]]), ALU.subtract)
        nc.sync.dma_start(fview(C[:], [[9, 32]], offset_elems=8), di["alpha_cols"][:])
        binr = sb.tile([1, 9], F32, tag="binr")
        nc.sync.dma_start(binr[:], di["alpha_row9"][:])
        # log_mu/log_nu constants
        LOGM = math.log(N1)
        LOGN0 = math.log(N0)
        u_m = pst.tile([128, 32], F32, tag="u_m")   # u main rows
        u_b = pst.tile([1, 1], F32, tag="u_b")      # u bin
        v_r = pst.tile([1, 9], F32, tag="v_r")      # v row
        nc.gpsimd.memset(u_m[:], 0.0)
        nc.gpsimd.memset(u_b[:], 0.0)
        nc.gpsimd.memset(v_r[:], 0.0)
        for it in range(OT_ITERS):
            # ---- u update: u = log_mu - LSE_over_ik9(C + v)
            ps_vb = ps.tile([128, 512], F32, tag="psA")
            K.mm(ps_vb[:, 0:9], ones[0:1, 0:128], v_r[:], tp=(0, 0))
            vb = sb.tile([128, 9], F32, tag="vb")
            nc.vector.tensor_copy(vb[:], ps_vb[:, 0:9])
            T = sb.tile([128, 288], F32, tag="T")
            nc.vector.tensor_tensor(T[:], C[:], fview(vb[:], [[0, 32], [1, 9]]), ALU.add)
            Te = sb.tile([128, 288], F32, tag="Te")
            nc.scalar.activation(Te[:], T[:], AF.Exp)
            rs = sb.tile([128, 32], F32, tag="rs")
            nc.vector.tensor_reduce(rs[:], fview(Te[:], [[9, 32], [1, 9]]), mybir.AxisListType.X, ALU.add)
            lnr = sb.tile([128, 32], F32, tag="lnr")
            nc.scalar.activation(lnr[:], rs[:], AF.Ln)
            nc.vector.tensor_scalar(u_m[:], lnr[:], -1.0, NORM, ALU.mult, ALU.add)
            # bin row u
            tb = sb.tile([1, 9], F32, tag="tb")
            nc.vector.tensor_tensor(tb[:], binr[:], v_r[:], ALU.add)
            tbe = sb.tile([1, 9], F32, tag="tbe")
            nc.scalar.activation(tbe[:], tb[:], AF.Exp)
            sb1 = sb.tile([1, 1], F32, tag="sb1")
            nc.vector.tensor_reduce(sb1[:], tbe[:], mybir.AxisListType.X, ALU.add)
            lb1 = sb.tile([1, 1], F32, tag="lb1")
            nc.scalar.activation(lb1[:], sb1[:], AF.Ln)
            nc.vector.tensor_scalar(u_b[:], lb1[:], -1.0, LOGN0 + NORM, ALU.mult, ALU.add)
            # ---- v update: v = log_nu - LSE_over_rows(C + u)
            CU = sb.tile([128, 288], F32, tag="T")
            nc.vector.tensor_tensor(CU[:], C[:], fview(u_m[:], [[1, 32], [0, 9]]), ALU.add)
            CUe = sb.tile([128, 288], F32, tag="Te")
            nc.scalar.activation(CUe[:], CU[:], AF.Exp)
            ps_cs2 = ps.tile([128, 512], F32, tag="psB")
            K.mm(ps_cs2[0:1, 0:288], ones[:, 0:1], CUe[:])
            cs2 = sb.tile([1, 288], F32, tag="cs2")
            nc.vector.tensor_copy(cs2[:], ps_cs2[0:1, 0:288])
            s9 = sb.tile([1, 9], F32, tag="s9")
            nc.vector.tensor_reduce(s9[:], fview(cs2[:], [[1, 9], [9, 32]]), mybir.AxisListType.X, ALU.add)
            # add bin-row exp(binr + u_b)
            tb2 = sb.tile([1, 9], F32, tag="tb")
            nc.vector.tensor_scalar_add(tb2[:], binr[:], u_b[0:1, 0:1])
            tbe2 = sb.tile([1, 9], F32, tag="tbe2")
            nc.scalar.activation(tbe2[:], tb2[:], AF.Exp)
            s9f = sb.tile([1, 9], F32, tag="s9f")
            nc.vector.tensor_tensor(s9f[:], s9[:], tbe2[:], ALU.add)
            l9 = sb.tile([1, 9], F32, tag="l9")
            nc.scalar.activation(l9[:], s9f[:], AF.Ln)
            nc.vector.tensor_scalar(v_r[:], l9[:], -1.0, NORM, ALU.mult, ALU.add)
            # log_nu last entry is log(m)+norm: fix bin col
            nc.vector.tensor_scalar_add(v_r[0:1, 8:9], v_r[0:1, 8:9], LOGM)

        # final scores F = C + u + v - norm ; output row = F[row0, 0:8]
        ps_vb = ps.tile([128, 512], F32, tag="psA")
        K.mm(ps_vb[:, 0:9], ones[0:1, 0:128], v_r[:], tp=(0, 0))
        vb = sb.tile([128, 9], F32, tag="vb")
        nc.vector.tensor_copy(vb[:], ps_vb[:, 0:9])
        Ff = sb.tile([128, 288], F32, tag="Ff")
        nc.vector.tensor_tensor(Ff[:], C[:], fview(vb[:], [[0, 32], [1, 9]]), ALU.add)
        nc.vector.tensor_tensor(Ff[:], Ff[:], fview(u_m[:], [[1, 32], [0, 9]]), ALU.add)
        nc.vector.tensor_scalar_add(Ff[:], Ff[:], -NORM)
        # min/max over cols (exclude ik9==8)
        mn = sb.tile([128, 2], F32, tag="mn")
        nc.vector.tensor_reduce(mn[:, 0:1], fview(Ff[:], [[9, 32], [1, 8]]), mybir.AxisListType.XY, ALU.min)
        nc.vector.tensor_reduce(mn[:, 1:2], fview(Ff[:], [[9, 32], [1, 8]]), mybir.AxisListType.XY, ALU.max)
        ps_tm = ps.tile([128, 512], F32, tag="psB")
        nc.tensor.transpose(ps_tm[0:2, 0:128], mn[:], ident[:])
        mnT = sb.tile([2, 128], F32, tag="mnT")
        nc.vector.tensor_copy(mnT[:], ps_tm[0:2, 0:128])
        mm2 = sb.tile([1, 1], F32, tag="mm2")
        nc.vector.tensor_reduce(mm2[:], mnT[0:1, :], mybir.AxisListType.X, ALU.min)
        mxrow = sb.tile([1, 128], F32, tag="mxrow")
        nc.sync.dma_start(mxrow[:], mnT[1:2, :])
        mx0 = sb.tile([1, 1], F32, tag="mx0")
        nc.vector.tensor_reduce(mx0[:], mxrow[:], mybir.AxisListType.X, ALU.max)
        smin = sb.tile([1, 1], F32, tag="smin")
        nc.vector.tensor_tensor(smin[:], mm2[:], mx0[:], ALU.subtract)
        nc.vector.tensor_scalar_add(smin[:], smin[:], -40.0)
        # row0 = C[0, 0:8] + u[0,0] + v[0:8] - norm
        row0 = sb.tile([1, 8], F32, tag="row0")
        nc.vector.tensor_scalar_add(row0[:], C[0:1, 0:8], u_m[0:1, 0:1])
        nc.vector.tensor_tensor(row0[:], row0[:], v_r[0:1, 0:8], ALU.add)
        nc.vector.tensor_scalar_add(row0[:], row0[:], -NORM)
        # out = (row0 + smin*maskf) * 15
        tmp8 = sb.tile([1, 8], F32, tag="tmp8")
        nc.vector.tensor_scalar(tmp8[:], maskf[:], smin[0:1, 0:1], None, ALU.mult)
        outr = sb.tile([1, 8], F32, tag="outr")
        nc.vector.tensor_tensor(outr[:], row0[:], tmp8[:], ALU.add)
        nc.vector.tensor_scalar_mul(outr[:], outr[:], 15.0)
        nc.sync.dma_start(out_t[:], outr[:])

        if DBG:
            nc.sync.dma_start(dbg["dbg_desc0"][:], d0[:])
            nc.sync.dma_start(dbg["dbg_desc2"][:], d2[:])
            nc.sync.dma_start(dbg["dbg_desc3"][:], d3[:])
            nc.sync.dma_start(dbg["dbg_d1loc"][:], d1l[:])
            nc.sync.dma_start(dbg["dbg_z"][:], zt[:])
            nc.sync.dma_start(dbg["dbg_row0"][0:1, 0:8], row0[:])
            nc.sync.dma_start(dbg["dbg_row0"][0:1, 8:9], smin[:])
        for p in (K.dram, K.ps, K.pst, K.sb1, K.sb, K.w):
            p.release()
    return nc


_CACHE = {}


def _get_nc(DBG=False):
    key = DBG
    if key not in _CACHE:
        nc = bacc.Bacc("TRN2", target_bir_lowering=False, debug=False, num_devices=NCORES)
        build(nc, DBG)
        nc.compile()
        _CACHE[key] = nc
    return _CACHE[key]


def make_in_maps(inputs):
    params = inputs["params"]
    flat = pack_params(params)
    d0 = _np(inputs["desc0"])[0]
    d1 = _np(inputs["desc1"])[0]
    d2 = _np(inputs["desc2"])[0]
    d3 = _np(inputs["desc3"])[0]
    dist = _np(inputs["dist"])[0]  # (128, 8, 4096)
    maskf = (~np.asarray(inputs["mask"])).astype(np.float32).reshape(1, N0)
    av = float(np.asarray(params["bin_score"]).reshape(()))
    alpha_cols = np.full((D, 32), av, np.float32)
    alpha_row9 = np.full((1, 9), av, np.float32)
    ident = np.eye(D, dtype=np.float32)

    common = dict(desc0=d0, desc1=d1, desc2=d2, desc3=d3, maskf=maskf,
                  alpha_cols=alpha_cols, alpha_row9=alpha_row9,
                  ident=ident, **flat)
    in_maps = []
    for s in range(NCORES):
        m = dict(common)
        m["desc1_loc"] = np.ascontiguousarray(d1[:, NS * s : NS * s + NS])
        m["dist0_loc"] = np.ascontiguousarray(
            dist[:, :, NS * s : NS * s + NS].reshape(D, 8 * NS))
        # dist1 cols for d_lo = s: dist1_loc[ch=(i0,d_hi), i1] = dist[d_hi*8+s, i0, i1]
        m["dist1_loc"] = np.ascontiguousarray(
            dist[s::8, :, :].transpose(1, 0, 2).reshape(D, N1))
        in_maps.append(m)
    return in_maps


def kernel(desc0, desc1, desc2, desc3, dist, params, mask, _dbg=False, _trace=False):
    in_maps = make_in_maps(dict(desc0=desc0, desc1=desc1, desc2=desc2, desc3=desc3,
                                dist=dist, params=params, mask=mask))
    nc = _get_nc(_dbg)
    res = bass_utils.run_bass_kernel_spmd(nc, in_maps, core_ids=list(range(NCORES)),
                                          trace=_trace)
    out = res.results[0]["out"].reshape(1, N0).astype(np.float32)
    if _dbg or _trace:
        return out, res
    return out


# revision 27
# speedup vs baseline: 1.3749x; 1.3749x over previous
"""Trainium2 Bass kernel for nn_AttentionalGNN (6-layer self/cross GNN + Sinkhorn).

Sharding: desc1's 4096 tokens are split into 8 contiguous blocks of 512 (one
per core).  Small tensors (desc0/desc2/desc3) and all weights are replicated;
heavy ops (desc1 self-attention, pair-MLP cross attention, desc1-side MLPs)
are computed on the owned block only.  One AllGather per layer rebuilds the
full desc1 (plus, on cross layers, partial-softmax partials for the ops whose
keys are sharded).  Softmaxes run unstabilized (score ranges verified small)
in a transposed [key-part, query-free] layout so no transposes are needed in
the hot loop.
"""

import math
import numpy as np

import concourse.bass as bass
import concourse.bacc as bacc
import concourse.mybir as mybir
import concourse.tile as tile
from concourse import bass_utils

F32 = mybir.dt.float32
AF = mybir.ActivationFunctionType
ALU = mybir.AluOpType

NCORES = 8
D = 128
N0, N1, N2, N3 = 8, 4096, 256, 256
H, DH = 4, 32
NS = N1 // NCORES  # 512
LAYER_NAMES = ["self", "cross", "self", "cross", "self", "cross"]
ISQ = 1.0 / math.sqrt(DH)
NORM = -math.log(N1 + N0)  # -log(m+n), m=4096 n=8
OT_ITERS = 5
DEBUG = False

# head-channel permutation: ours c' = h*32+d  <-  reference c = d*4+h
PERM = np.array([(c % DH) * H + c // DH for c in range(D)])

# cross-layer payload row map
PR_D1 = 0        # rows 0:128   desc1 block (cols 0:512)
PR_M21 = 128     # rows 128:256 cols 0:256 msg21 partial; cols 256:264 msg0 partial; row 128 cols 272:280 l0 partial
PR_L21 = 256     # rows 256:384 l21 partial (rows 32h used, cols 0:256)
PR_S1 = 384      # rows 384:512 score1 raw [128, 32] (layer 5 only)
R_CROSS = 384
R_CROSS5 = 512
R_SELF = 128


def _np(x):
    return np.asarray(x, dtype=np.float32)


def _pack_mha(p):
    (wq, bq), (wk, bk), (wv, bv) = [(_np(w), _np(b)) for w, b in p["proj"]]
    wm, bm = _np(p["merge"][0]), _np(p["merge"][1])
    out = {
        "wqT": wq[PERM, :].T.copy(),
        "bq": bq[PERM].reshape(D, 1).copy(),
        "wkT": wk[PERM, :].T.copy(),
        "bk": bk[PERM].reshape(D, 1).copy(),
        "wvT": wv[PERM, :].T.copy(),
        "wmT": wm[:, PERM].T.copy(),
        "bm": (wm @ bv + bm).reshape(D, 1).copy(),
    }
    return out


def _pack_mlp(p):
    (w1, b1), (w2, b2) = [(_np(w), _np(b)) for w, b in p]
    w1T = w1.T  # (2D, 2D)
    w2T = w2.T  # (2D, D)
    return {
        "w1Ta": w1T[0:D, :].copy(),
        "w1Tb": w1T[D : 2 * D, :].copy(),
        "b1a": b1[0:D].reshape(D, 1).copy(),
        "b1b": b1[D : 2 * D].reshape(D, 1).copy(),
        "w2Ta": w2T[0:D, :].copy(),
        "w2Tb": w2T[D : 2 * D, :].copy(),
        "b2": b2.reshape(D, 1).copy(),
    }


def _pack_xattn(p):
    (w1, b1), (w2, b2) = [(_np(w), _np(b)) for w, b in p["mlp"]]
    w1T = w1.T  # (3D, D)
    return {
        "w1qT": w1T[0:D, :].copy(),
        "w1kT": w1T[D : 2 * D, :].copy(),
        "w1dT": w1T[2 * D : 3 * D, :].copy(),
        "b1": b1.reshape(D, 1).copy(),
        "w2T": w2.T.copy(),  # (D, 1)
    }


def pack_params(params):
    flat = {}
    for i, name in enumerate(LAYER_NAMES):
        ap, php, ghp = params["attn"][i], params["phattn"][i], params["ghattn"][i]
        if name == "cross":
            for k, v in _pack_xattn(ap["attn"]).items():
                flat[f"L{i}_a_{k}"] = v
        else:
            for k, v in _pack_mha(ap["attn"]).items():
                flat[f"L{i}_a_{k}"] = v
        for k, v in _pack_mlp(ap["mlp"]).items():
            flat[f"L{i}_am_{k}"] = v
        for k, v in _pack_mha(php["attn"]).items():
            flat[f"L{i}_p_{k}"] = v
        for k, v in _pack_mlp(php["mlp"]).items():
            flat[f"L{i}_pm_{k}"] = v
        for k, v in _pack_mha(ghp["attn"]).items():
            flat[f"L{i}_g_{k}"] = v
        for k, v in _pack_mlp(ghp["mlp"]).items():
            flat[f"L{i}_gm_{k}"] = v
    return flat


def fview(ap, free_dims, offset_elems=0):
    """Replace the free dims of a [P, ...] AP (keeps the partition dim)."""
    return bass.AP(ap.tensor, ap.offset + offset_elems, [list(ap.ap[0])] + [list(d) for d in free_dims])


class Kern:
    def __init__(self, nc, tc):
        self.nc = nc
        self.tc = tc
        self.w = tc.alloc_tile_pool(name="w", bufs=2)
        self.sb = tc.alloc_tile_pool(name="sb", bufs=2)
        self.sb1 = tc.alloc_tile_pool(name="sb1", bufs=1)
        self.pst = tc.alloc_tile_pool(name="pst", bufs=1)  # persistent state
        self.ps = tc.alloc_tile_pool(name="ps", bufs=1, space="PSUM")
        self.dram = tc.alloc_tile_pool(name="dram", bufs=1, space="DRAM")
        self.wt = {}

    # ---------- weights ----------
    def layer_weights(self, li, wspec_layer, tensors):
        wt = {}
        for name, shape in wspec_layer:
            field = name.split("_", 1)[1]  # strip L{i}_ prefix
            t = self.w.tile(list(shape), F32, tag=f"w_{field}")
            self.nc.sync.dma_start(t[:], tensors[name][:])
            wt[field] = t
        return wt

    # ---------- primitives ----------
    def mm(self, psum_ap, lhsT, rhs, start=True, stop=True, tp=None):
        self.nc.tensor.matmul(psum_ap, lhsT, rhs, start=start, stop=stop,
                              tile_position=tp, skip_group_check=True)

    def evac_act(self, out_ap, psum_ap, bias=None):
        if bias is None:
            self.nc.scalar.activation(out_ap, psum_ap, AF.Identity)
        else:
            self.nc.scalar.activation(out_ap, psum_ap, AF.Identity, bias=bias)

    def evac(self, out_ap, psum_ap, bias=None, relu=False):
        v = self.nc.vector
        if bias is not None and relu:
            v.tensor_scalar(out_ap, psum_ap, bias, 0.0, ALU.add, ALU.max)
        elif bias is not None:
            v.tensor_scalar_add(out_ap, psum_ap, bias)
        elif relu:
            v.tensor_scalar_max(out_ap, psum_ap, 0.0)
        else:
            v.tensor_copy(out_ap, psum_ap)

    def bcast_rows(self, ps_out, row_ap, base, nrows):
        """row at partition `base` -> ps_out[base:base+nrows, :] via PE outer product."""
        ones = self.ones
        n = row_ap.ap[-1][1]
        self.mm(ps_out, ones[base : base + 1, 0:nrows], row_ap, tp=((base // 32) * 32, (base // 32) * 32))

    # ---------- MHA on a query block ----------
    def mha(self, tag, xq, nq, kv, nk, W, partial=False):
        nc, sb, ps = self.nc, self.sb, self.ps
        nkt = (nk + 127) // 128

        psq = ps.tile([128, 512], F32, tag="psA")
        self.mm(psq[:, 0:nq], W["wqT"][:], xq)
        qh = sb.tile([128, 512], mybir.dt.bfloat16, tag="qh")
        self.evac_act(qh[:, 0:nq], psq[:, 0:nq], bias=W["bq"][:, 0:1])

        ps_msg = ps.tile([128, 512], F32, tag="psMSG")
        ps_l = ps.tile([128, 512], F32, tag="psL")
        esum = []
        for h in range(H):
            t_esum = self.sb1.tile([128, nq], F32, tag=f"esum{h}")
            esum.append(t_esum)

        for kt in range(nkt):
            kk = min(128, nk - kt * 128)
            ksl = slice(kt * 128, kt * 128 + kk)
            # kh tile [128, kk]
            ps_kh = ps.tile([128, 512], F32, tag="psB")
            self.mm(ps_kh[:, 0:kk], W["wkT"][:], kv[:, ksl])
            kh = sb.tile([128, 128], mybir.dt.bfloat16, tag="kh")
            self.evac_act(kh[:, 0:kk], ps_kh[:, 0:kk], bias=W["bk"][:, 0:1])
            # vT tile [kk, 128]
            ps_vT = ps.tile([128, 512], F32, tag="psL")
            self.mm(ps_vT[0:kk, 0:128], kv[:, ksl], W["wvT"][:])
            vT = sb.tile([128, 128], mybir.dt.bfloat16, tag="vT")
            self.evac_act(vT[0:kk, :], ps_vT[0:kk, 0:128])
            first, last = kt == 0, kt == nkt - 1
            for h in range(H):
                hs = slice(32 * h, 32 * h + 32)
                ps_s = ps.tile([128, 512], F32, tag=f"psS{h}")
                self.mm(ps_s[0:kk, 0:nq], kh[hs, 0:kk], qh[hs, 0:nq], tp=(32 * h, 0))
                es = sb.tile([128, nq], mybir.dt.bfloat16, tag=f"es{h}")
                nc.scalar.activation(es[0:kk, 0:nq], ps_s[0:kk, 0:nq], AF.Exp, scale=ISQ)
                self.mm(ps_msg[hs, 0:nq], vT[0:kk, hs], es[0:kk, 0:nq],
                        start=first, stop=last, tp=(0, 32 * h))
                if first:
                    nc.vector.tensor_copy(esum[h][0:kk, :], es[0:kk, 0:nq])
                else:
                    nc.vector.tensor_tensor(esum[h][:], esum[h][:], es[:, 0:nq], ALU.add)
        kfin = min(128, nk)
        for h in range(H):
            self.mm(ps_l[32 * h : 32 * h + 1, 0:nq], self.ones[0:kfin, 0:1],
                    esum[h][0:kfin, :], start=True, stop=True, tp=(0, 32 * h))

        if partial:
            msg_sb = sb.tile([128, nq], F32, tag="pmsg")
            l_sb = sb.tile([128, nq], F32, tag="pl")
            self.evac(msg_sb[:], ps_msg[:, 0:nq])
            self.evac(l_sb[:], ps_l[:, 0:nq])
            return msg_sb, l_sb

        msgn = self.normalize_msg(ps_msg, ps_l, nq)
        # merge conv
        ps_m = ps.tile([128, 512], F32, tag="psA")
        self.mm(ps_m[:, 0:nq], W["wmT"][:], msgn[:, 0:nq])
        attn = sb.tile([128, nq], F32, tag=tag)
        self.evac(attn[:], ps_m[:, 0:nq], bias=W["bm"][:, 0:1])
        return attn

    def normalize_msg(self, ps_msg, ps_l, nq, msg_sb=None, l_sb=None):
        """msg/l either in PSUM (ps_msg/ps_l) or SBUF; returns normalized msg [128, nq] SBUF."""
        nc, sb, ps = self.nc, self.sb, self.ps
        src_l = ps_l if l_sb is None else l_sb
        recip = sb.tile([128, nq], F32, tag="recip")
        for h in range(H):
            r = slice(32 * h, 32 * h + 1)
            nc.vector.reciprocal(recip[r, 0:nq], src_l[r, 0:nq])
        ps_bc = ps.tile([128, 512], F32, tag="psB")
        for h in range(H):
            self.bcast_rows(ps_bc[32 * h : 32 * h + 32, 0:nq], recip[32 * h : 32 * h + 1, 0:nq], 32 * h, 32)
        msgn = sb.tile([128, nq], F32, tag="msgn")
        if msg_sb is None:
            nc.vector.tensor_copy(msgn[:], ps_msg[:, 0:nq])
            nc.vector.tensor_tensor(msgn[:], msgn[:], ps_bc[:, 0:nq], ALU.mult)
        else:
            nc.vector.tensor_tensor(msgn[:], msg_sb[:, 0:nq], ps_bc[:, 0:nq], ALU.mult)
        return msgn

    # ---------- two-layer prop MLP on [x; attn] ----------
    def prop_mlp(self, tag, x, attn, M, nq, out_ap=None, out_bias_add=None):
        nc, sb, ps = self.nc, self.sb, self.ps
        hs = []
        for m in range(2):
            msl = slice(128 * m, 128 * m + 128)
            ph = ps.tile([128, 512], F32, tag="psA")
            self.mm(ph[:, 0:nq], M["w1Ta"][:, msl], x, start=True, stop=False)
            self.mm(ph[:, 0:nq], M["w1Tb"][:, msl], attn, start=False, stop=True)
            hm = sb.tile([128, nq], F32, tag=f"h{m}")
            self.evac(hm[:], ph[:, 0:nq], bias=M["b1a" if m == 0 else "b1b"][:, 0:1], relu=True)
            hs.append(hm)
        pd = ps.tile([128, 512], F32, tag="psB")
        self.mm(pd[:, 0:nq], M["w2Ta"][:], hs[0][:], start=True, stop=False)
        self.mm(pd[:, 0:nq], M["w2Tb"][:], hs[1][:], start=False, stop=True)
        if out_ap is None:
            delta = sb.tile([128, nq], F32, tag=tag)
            self.evac(delta[:], pd[:, 0:nq], bias=M["b2"][:, 0:1])
            return delta
        else:
            self.evac(out_ap, pd[:, 0:nq], bias=M["b2"][:, 0:1])
            return None


def build(nc, DBG):
    di = {}

    def inp(name, shape):
        di[name] = nc.dram_tensor(name, list(shape), F32, kind="ExternalInput").ap()
        return di[name]

    inp("desc0", (D, N0)); inp("desc1", (D, N1)); inp("desc2", (D, N2)); inp("desc3", (D, N3))
    inp("desc1_loc", (D, NS)); inp("dist0_loc", (D, 8 * NS)); inp("dist1_loc", (D, 8 * NS))
    inp("maskf", (1, N0)); inp("alpha_cols", (D, 32)); inp("alpha_row9", (1, 9)); inp("ident", (D, D))
    out_t = nc.dram_tensor("out", [1, N0], F32, kind="ExternalOutput").ap()
    dbg = {}
    if DBG:
        for nm, shape in [("dbg_desc0", (D, N0)), ("dbg_desc2", (D, N2)), ("dbg_desc3", (D, N3)),
                          ("dbg_d1loc", (D, NS)), ("dbg_z", (D, 256)), ("dbg_row0", (1, 16))]:
            dbg[nm] = nc.dram_tensor(nm, list(shape), F32, kind="ExternalOutput").ap()

    wspec = []
    wspec_by_layer = []
    for i, name in enumerate(LAYER_NAMES):
        w0 = len(wspec)
        if name == "cross":
            wspec += [(f"L{i}_a_w1qT", (D, D)), (f"L{i}_a_w1kT", (D, D)), (f"L{i}_a_w1dT", (D, D)),
                      (f"L{i}_a_b1", (D, 1)), (f"L{i}_a_w2T", (D, 1))]
        else:
            wspec += [(f"L{i}_a_{k}", (D, D)) for k in ["wqT", "wkT", "wvT", "wmT"]]
            wspec += [(f"L{i}_a_{k}", (D, 1)) for k in ["bq", "bk", "bm"]]
        for g in ["am", "pm", "gm"]:
            wspec += [(f"L{i}_{g}_w1Ta", (D, 2 * D)), (f"L{i}_{g}_w1Tb", (D, 2 * D)),
                      (f"L{i}_{g}_w2Ta", (D, D)), (f"L{i}_{g}_w2Tb", (D, D))]
            wspec += [(f"L{i}_{g}_b1a", (D, 1)), (f"L{i}_{g}_b1b", (D, 1)), (f"L{i}_{g}_b2", (D, 1))]
        for g in ["p", "g"]:
            wspec += [(f"L{i}_{g}_{k}", (D, D)) for k in ["wqT", "wkT", "wvT", "wmT"]]
            wspec += [(f"L{i}_{g}_{k}", (D, 1)) for k in ["bq", "bk", "bm"]]
        wspec_by_layer.append(wspec[w0:])
    for n, s in wspec:
        di[n] = nc.dram_tensor(n, list(s), F32, kind="ExternalInput").ap()

    with tile.TileContext(nc) as tc:
        K = Kern(nc, tc)
        sb, ps, pst, dram = K.sb, K.ps, K.pst, K.dram

        ones = K.w.tile([128, 128], F32, tag="ones")
        nc.gpsimd.memset(ones[:], 1.0)
        K.ones = ones
        ident = K.w.tile([128, 128], F32, tag="ident")
        nc.sync.dma_start(ident[:], di["ident"][:])
        maskf = K.w.tile([1, N0], F32, tag="maskf")
        nc.sync.dma_start(maskf[:], di["maskf"][:])


        # persistent state
        d0 = pst.tile([D, N0], F32, tag="d0")
        d2 = pst.tile([D, N2], F32, tag="d2")
        d3 = pst.tile([D, N3], F32, tag="d3")
        d1f = pst.tile([D, N1], F32, tag="d1f")
        d1l = pst.tile([D, NS], F32, tag="d1l")
        for t, n in [(d0, "desc0"), (d2, "desc2"), (d3, "desc3"), (d1f, "desc1"),
                     (d1l, "desc1_loc")]:
            nc.sync.dma_start(t[:], di[n][:])

        z_tile = [None]  # score1 container post layer 5

        LW = [None]

        def mw(i, g, k):
            return LW[0][f"{g}_{k}"]

        def Wd(i, g):
            return {k: mw(i, g, k) for k in ["wqT", "bq", "wkT", "bk", "wvT", "wmT", "bm"]}

        def Md(i, g):
            return {k: mw(i, g, k) for k in ["w1Ta", "w1Tb", "b1a", "b1b", "w2Ta", "w2Tb", "b2"]}

        def do_gather(li, payload_rows, writes):
            pay = dram.tile([payload_rows, 512], F32, tag=f"pay{li}")
            for (r, c, srcap) in writes:
                p = srcap.ap[0][1]
                nc.sync.dma_start(pay[r : r + p, c : c + srcap.free_size()], srcap)
            gath = dram.tile([payload_rows * 8, 512], F32, tag=f"gath{li}", addr_space="Shared")
            nc.gpsimd.collective_compute(
                "AllGather", ALU.bypass, replica_groups=[list(range(NCORES))],
                ins=[pay.opt()], outs=[gath.opt()])
            return gath

        def reload_d1f(gath, R):
            # gathered [R*8, 512]: core s rows [R*s : R*s+128] -> d1f[:, 512s:...]
            gap = gath[:]
            src = bass.AP(gap.tensor, gap.offset, [[512, 128], [R * 512, 8], [1, 512]])
            nc.sync.dma_start(fview(d1f[:], [[512, 8], [1, 512]]), src)

        # ============ layers ============
        for li, lname in enumerate(LAYER_NAMES):
            LW[0] = K.layer_weights(li, wspec_by_layer[li], di)
            if lname == "self":
                a1 = K.mha("attn", d1l[:], NS, d1f[:], N1, Wd(li, "a"))
                dl1 = K.prop_mlp("delta", d1l[:], a1[:], Md(li, "am"), NS)
                nc.vector.tensor_add(d1l[:], d1l[:], dl1[:])
                gath = do_gather(li, R_SELF, [(PR_D1, 0, d1l[:])])
                reload_d1f(gath, R_SELF)
                a0 = K.mha("attn", d0[:], N0, d0[:], N0, Wd(li, "a"))
                dl0 = K.prop_mlp("delta", d0[:], a0[:], Md(li, "am"), N0)
                nc.vector.tensor_add(d0[:], d0[:], dl0[:])
                a2 = K.mha("attn", d2[:], N2, d2[:], N2, Wd(li, "p"))
                dl2 = K.prop_mlp("delta", d2[:], a2[:], Md(li, "pm"), N2)
                nc.vector.tensor_add(d2[:], d2[:], dl2[:])
                a3 = K.mha("attn", d3[:], N3, d3[:], N3, Wd(li, "g"))
                dl3 = K.prop_mlp("delta", d3[:], a3[:], Md(li, "gm"), N3)
                nc.vector.tensor_add(d3[:], d3[:], dl3[:])
            else:
                # ---- delta0 partials (pair-MLP, queries=desc0, keys=my desc1 block) ----
                # transpose my desc1 block -> d1lT [i1l, c] (4 tiles of 128)
                d1lT = sb.tile([128, 512], F32, tag="d1lT")
                for t in range(4):
                    ps_t = ps.tile([128, 512], F32, tag="psB")
                    nc.tensor.transpose(ps_t[:, 0:128], d1l[:, 128 * t : 128 * t + 128], ident[:])
                    K.evac(d1lT[:, 128 * t : 128 * t + 128], ps_t[:, 0:128])
                es0 = sb.tile([N0, 512], F32, tag="es0")
                ps_sk = ps.tile([128, 512], F32, tag="psB")
                K.mm(ps_sk[:, 0:NS], mw(li, "a", "w1kT")[:], d1l[:])
                Sk = K.sb1.tile([128, NS], F32, tag="Sk")
                K.evac(Sk[:], ps_sk[:, 0:NS])
                ps_qb = ps.tile([128, 512], F32, tag="psB")
                K.mm(ps_qb[:, 0:N0], mw(li, "a", "w1qT")[:], d0[:])
                Qbb = sb.tile([128, N0], F32, tag="Qbb")
                K.evac(Qbb[:], ps_qb[:, 0:N0], bias=mw(li, "a", "b1")[:, 0:1])
                for i0 in range(N0):
                    dch0 = sb.tile([128, NS], F32, tag="dch")
                    nc.sync.dma_start(dch0[:], di["dist0_loc"][:, NS * i0 : NS * i0 + NS])
                    ph = ps.tile([128, 512], F32, tag="psA")
                    K.mm(ph[:, 0:NS], mw(li, "a", "w1dT")[:], dch0[:])
                    nc.vector.tensor_tensor(ph[:, 0:NS], ph[:, 0:NS], Sk[:], ALU.add)
                    hid = sb.tile([128, NS], F32, tag="hid0")
                    nc.scalar.activation(hid[:], ph[:, 0:NS], AF.Relu, bias=Qbb[:, i0 : i0 + 1])
                    ps_s = ps.tile([128, 512], F32, tag="psL")
                    K.mm(ps_s[0:1, 0:NS], mw(li, "a", "w2T")[:], hid[:])
                    er = sb.tile([1, NS], F32, tag="er0")
                    nc.scalar.activation(er[:], ps_s[0:1, 0:NS], AF.Exp)
                    nc.sync.dma_start(es0[i0 : i0 + 1, :], er[:])
                # transpose es0 [8, 512] -> es0T tiles [128, 8] x4 ; partial msg0/l0
                ps_m0 = ps.tile([128, 512], F32, tag="psS0")
                ps_l0 = ps.tile([128, 512], F32, tag="psS1")
                for t in range(4):
                    ps_t = ps.tile([128, 512], F32, tag="psB")
                    nc.tensor.transpose(ps_t[0:128, 0:N0], es0[0:N0, 128 * t : 128 * t + 128], ident[0:N0, 0:N0])
                    e0T = sb.tile([128, N0], F32, tag="e0T")
                    K.evac(e0T[:], ps_t[0:128, 0:N0])
                    K.mm(ps_m0[:, 0:N0], d1lT[:, 128 * t : 128 * t + 128], e0T[:],
                         start=(t == 0), stop=(t == 3))
                    K.mm(ps_l0[0:1, 0:N0], ones[:, 0:1], e0T[:], start=(t == 0), stop=(t == 3))
                m0p = sb.tile([128, N0], F32, tag="m0p")
                l0p = sb.tile([1, N0], F32, tag="l0p")
                K.evac(m0p[:], ps_m0[:, 0:N0])
                K.evac(l0p[:], ps_l0[0:1, 0:N0])

                # ---- delta1 (pair-MLP, queries=my block, keys=desc0) ----
                sc1 = K.sb1.tile([1, 8 * NS], F32, tag="sc1")
                ps_q1 = ps.tile([128, 512], F32, tag="psB")
                K.mm(ps_q1[:, 0:NS], mw(li, "a", "w1qT")[:], d1l[:])
                Qloc = K.sb1.tile([128, NS], F32, tag="Qloc")
                K.evac(Qloc[:], ps_q1[:, 0:NS])
                ps_k8 = ps.tile([128, 512], F32, tag="psB")
                K.mm(ps_k8[:, 0:N0], mw(li, "a", "w1kT")[:], d0[:])
                Kt8 = sb.tile([128, N0], F32, tag="Kt8")
                K.evac(Kt8[:], ps_k8[:, 0:N0])
                for c8 in range(8):
                    csl = slice(NS * c8, NS * c8 + NS)
                    dch1 = sb.tile([128, NS], F32, tag="dch")
                    nc.sync.dma_start(dch1[:], di["dist1_loc"][:, csl])
                    ph = ps.tile([128, 512], F32, tag="psA")
                    K.mm(ph[:, 0:NS], mw(li, "a", "w1dT")[:], dch1[:])
                    nc.vector.tensor_tensor(ph[:, 0:NS], ph[:, 0:NS],
                                            fview(Qloc[:], [[1, 64], [0, 8]], offset_elems=64 * c8), ALU.add)
                    nc.vector.tensor_tensor(ph[:, 0:NS], ph[:, 0:NS],
                                            fview(Kt8[:], [[0, 64], [1, 8]]), ALU.add)
                    hid = sb.tile([128, NS], F32, tag="hid1")
                    nc.scalar.activation(hid[:], ph[:, 0:NS], AF.Relu, bias=mw(li, "a", "b1")[:, 0:1])
                    ps_s = ps.tile([128, 512], F32, tag="psL")
                    K.mm(ps_s[0:1, 0:NS], mw(li, "a", "w2T")[:], hid[:])
                    nc.vector.tensor_copy(sc1[0:1, csl], ps_s[0:1, 0:NS])
                # reshape to S1t [128, (4 m, 8 ik)]  (iq_local = 4p + m)
                S1t = sb.tile([128, 32], F32, tag="S1t", )
                s1ap = sc1[:]
                src = bass.AP(s1ap.tensor, s1ap.offset, [[1, 1], [32, 128], [8, 4], [1, 8]])
                nc.sync.dma_start(fview(S1t[:], [[8, 4], [1, 8]]), src)
                eS1 = sb.tile([128, 32], F32, tag="eS1")
                nc.scalar.activation(eS1[:], S1t[:], AF.Exp)
                # transpose -> E1T [32, 128]; shift m-groups to partition 0
                ps_t = ps.tile([128, 512], F32, tag="psB")
                nc.tensor.transpose(ps_t[0:32, 0:128], eS1[:], ident[:])
                E1T = sb.tile([32, 128], F32, tag="E1T")
                K.evac(E1T[:], ps_t[0:32, 0:128])
                E1m = sb.tile([8, 4 * 128], F32, tag="E1m")
                for m in range(4):
                    nc.sync.dma_start(E1m[0:8, 128 * m : 128 * m + 128], E1T[8 * m : 8 * m + 8, :])
                # desc0^T
                ps_t0 = ps.tile([128, 512], F32, tag="psB")
                nc.tensor.transpose(ps_t0[0:N0, 0:128], d0[:], ident[:])
                d0T = sb.tile([N0, 128], F32, tag="d0T")
                K.evac(d0T[:], ps_t0[0:N0, 0:128])
                delta1 = sb.tile([128, NS], F32, tag="delta1")
                for m in range(4):
                    ps_m1 = ps.tile([128, 512], F32, tag="psS0")
                    K.mm(ps_m1[:, 0:128], d0T[:], E1m[0:8, 128 * m : 128 * m + 128])
                    ps_l1 = ps.tile([128, 512], F32, tag="psS1")
                    K.mm(ps_l1[0:1, 0:128], ones[0:8, 0:1], E1m[0:8, 128 * m : 128 * m + 128])
                    r1 = sb.tile([1, 128], F32, tag="r1")
                    nc.vector.reciprocal(r1[:], ps_l1[0:1, 0:128])
                    ps_bc = ps.tile([128, 512], F32, tag="psMSG")
                    K.mm(ps_bc[:, 0:128], ones[0:1, 0:128], r1[:], tp=(0, 0))
                    m1n = sb.tile([128, 128], F32, tag="m1n")
                    nc.vector.tensor_copy(m1n[:], ps_m1[:, 0:128])
                    nc.vector.tensor_tensor(m1n[:], m1n[:], ps_bc[:, 0:128], ALU.mult)
                    xv = fview(d1l[:], [[4, 128]], offset_elems=m)
                    ov = fview(delta1[:], [[4, 128]], offset_elems=m)
                    K.prop_mlp("pd1", xv, m1n[:], Md(li, "am"), 128, out_ap=ov)

                # ---- delta12 / delta21 / ghp ----
                a12 = K.mha("attn", d1l[:], NS, d2[:], N2, Wd(li, "p"))
                dl12 = K.prop_mlp("dl12", d1l[:], a12[:], Md(li, "pm"), NS)
                m21, l21 = K.mha("a21", d2[:], N2, d1l[:], NS, Wd(li, "p"), partial=True)
                # ---- updates (desc1 block) after all readers of old d1l ----
                nc.vector.tensor_add(d1l[:], d1l[:], delta1[:])
                nc.vector.tensor_add(d1l[:], d1l[:], dl12[:])

                # ---- gather (overlaps with ghp props below) ----
                R = R_CROSS5 if li == 5 else R_CROSS
                writes = [(PR_D1, 0, d1l[:]), (PR_M21, 0, m21[:]), (PR_M21, 256, m0p[:]),
                          (PR_L21, 0, l21[:])]
                if li == 5:
                    writes.append((PR_S1, 0, S1t[:]))
                pay = dram.tile([R, 512], F32, tag=f"pay{li}")
                for (r, c, srcap) in writes:
                    p = srcap.ap[0][1]
                    nc.sync.dma_start(pay[r : r + p, c : c + srcap.free_size()], srcap)
                nc.sync.dma_start(pay[PR_M21 : PR_M21 + 1, 272:280], l0p[:])
                gath = dram.tile([R * 8, 512], F32, tag=f"gath{li}", addr_space="Shared")
                nc.gpsimd.collective_compute(
                    "AllGather", ALU.bypass, replica_groups=[list(range(NCORES))],
                    ins=[pay.opt()], outs=[gath.opt()])
                reload_d1f(gath, R)

                a03 = K.mha("attn", d0[:], N0, d3[:], N3, Wd(li, "g"))
                dl03 = K.prop_mlp("dl03", d0[:], a03[:], Md(li, "gm"), N0)
                a30 = K.mha("attn", d3[:], N3, d0[:], N0, Wd(li, "g"))
                dl30 = K.prop_mlp("delta", d3[:], a30[:], Md(li, "gm"), N3)
                nc.vector.tensor_add(d3[:], d3[:], dl30[:])

                # ---- post-gather: sum partials ----
                gap = gath[:]
                acc1 = sb.tile([128, 512], F32, tag="acc1")
                acc2 = sb.tile([128, 256], F32, tag="acc2")
                for s in range(8):
                    t1 = sb.tile([128, 512], F32, tag="gtmp1")
                    src = bass.AP(gap.tensor, gap.offset + (R * s + PR_M21) * 512, [[512, 128], [1, 512]])
                    nc.sync.dma_start(t1[:], src)
                    if s == 0:
                        nc.vector.tensor_copy(acc1[:], t1[:])
                    else:
                        nc.vector.tensor_add(acc1[:], acc1[:], t1[:])
                    t2 = sb.tile([128, 256], F32, tag="gtmp2")
                    src2 = bass.AP(gap.tensor, gap.offset + (R * s + PR_L21) * 512, [[512, 128], [1, 256]])
                    nc.sync.dma_start(t2[:], src2)
                    if s == 0:
                        nc.vector.tensor_copy(acc2[:], t2[:])
                    else:
                        nc.vector.tensor_add(acc2[:], acc2[:], t2[:])
                # delta21: normalize msg21 (acc1 cols 0:256, l in acc2) + merge + mlp
                m21n = K.normalize_msg(None, None, N2, msg_sb=acc1, l_sb=acc2)
                ps_m = ps.tile([128, 512], F32, tag="psA")
                K.mm(ps_m[:, 0:N2], mw(li, "p", "wmT")[:], m21n[:, 0:N2])
                a21 = sb.tile([128, N2], F32, tag="a21f")
                K.evac(a21[:], ps_m[:, 0:N2], bias=mw(li, "p", "bm")[:, 0:1])
                dl21 = K.prop_mlp("delta", d2[:], a21[:], Md(li, "pm"), N2)
                # delta0: normalize msg0 (acc1 cols 256:264 / l row at [0:1, 272:280])
                r0 = sb.tile([1, N0], F32, tag="r0")
                nc.vector.reciprocal(r0[:], acc1[0:1, 272:280])
                ps_bc0 = ps.tile([128, 512], F32, tag="psB")
                K.mm(ps_bc0[:, 0:N0], ones[0:1, 0:128], r0[:], tp=(0, 0))
                m0n = sb.tile([128, N0], F32, tag="m0n")
                nc.vector.tensor_tensor(m0n[:], acc1[:, 256:264], ps_bc0[:, 0:N0], ALU.mult)
                nc.vector.tensor_add(d2[:], d2[:], dl21[:])
                dl0 = K.prop_mlp("delta", d0[:], m0n[:], Md(li, "am"), N0)
                nc.vector.tensor_add(d0[:], d0[:], dl0[:])
                nc.vector.tensor_add(d0[:], d0[:], dl03[:])

                if li == 5:
                    # Z [128, (8 s, 4 m, 8 ik)] raw score1
                    zt = pst.tile([128, 256], F32, tag="Z")
                    src = bass.AP(gap.tensor, gap.offset + PR_S1 * 512,
                                  [[512, 128], [R * 512, 8], [8, 4], [1, 8]])
                    nc.sync.dma_start(fview(zt[:], [[32, 8], [8, 4], [1, 8]]), src)
                    z_tile[0] = zt

        # ============ Sinkhorn + output ============
        zt = z_tile[0]
        # log_softmax over rows (per ik): E=exp(Z); colsum via ones-matmul; LSE=ln(32*mean)... use sum directly
        E = sb.tile([128, 256], F32, tag="E")
        nc.scalar.activation(E[:], zt[:], AF.Exp)
        ps_cs = ps.tile([128, 512], F32, tag="psA")
        K.mm(ps_cs[0:1, 0:256], ones[:, 0:1], E[:])
        csr = sb.tile([1, 256], F32, tag="csr")
        nc.vector.tensor_copy(csr[:], ps_cs[0:1, 0:256])
        sum8 = sb.tile([1, 8], F32, tag="sum8")
        nc.vector.tensor_reduce(sum8[:], fview(csr[:], [[1, 8], [8, 32]]), mybir.AxisListType.X, ALU.add)
        lse8 = sb.tile([1, 8], F32, tag="lse8")
        nc.scalar.activation(lse8[:], sum8[:], AF.Ln)
        ps_lb = ps.tile([128, 512], F32, tag="psB")
        K.mm(ps_lb[:, 0:8], ones[0:1, 0:128], lse8[:], tp=(0, 0))
        lseb = sb.tile([128, 8], F32, tag="lseb")
        nc.vector.tensor_copy(lseb[:], ps_lb[:, 0:8])
        # couplings C [128, (32 t, 9)]
        C = pst.tile([128, 288], F32, tag="C")
        zls_v = fview(C[:], [[9, 32], [1, 8]])
        nc.vector.tensor_tensor(zls_v, zt[:], fview(lseb[:], [[0, 32], [1, 8]]), ALU.subtract)
        nc.sync.dma_start(fview(C[:], [[9, 32]], offset_elems=8), di["alpha_cols"][:])
        binr = sb.tile([1, 9], F32, tag="binr")
        nc.sync.dma_start(binr[:], di["alpha_row9"][:])
        # log_mu/log_nu constants
        LOGM = math.log(N1)
        LOGN0 = math.log(N0)
        u_m = pst.tile([128, 32], F32, tag="u_m")   # u main rows
        u_b = pst.tile([1, 1], F32, tag="u_b")      # u bin
        v_r = pst.tile([1, 9], F32, tag="v_r")      # v row
        nc.gpsimd.memset(u_m[:], 0.0)
        nc.gpsimd.memset(u_b[:], 0.0)
        nc.gpsimd.memset(v_r[:], 0.0)
        for it in range(OT_ITERS):
            # ---- u update: u = log_mu - LSE_over_ik9(C + v)
            ps_vb = ps.tile([128, 512], F32, tag="psA")
            K.mm(ps_vb[:, 0:9], ones[0:1, 0:128], v_r[:], tp=(0, 0))
            vb = sb.tile([128, 9], F32, tag="vb")
            nc.vector.tensor_copy(vb[:], ps_vb[:, 0:9])
            T = sb.tile([128, 288], F32, tag="T")
            nc.vector.tensor_tensor(T[:], C[:], fview(vb[:], [[0, 32], [1, 9]]), ALU.add)
            Te = sb.tile([128, 288], F32, tag="Te")
            nc.scalar.activation(Te[:], T[:], AF.Exp)
            rs = sb.tile([128, 32], F32, tag="rs")
            nc.vector.tensor_reduce(rs[:], fview(Te[:], [[9, 32], [1, 9]]), mybir.AxisListType.X, ALU.add)
            lnr = sb.tile([128, 32], F32, tag="lnr")
            nc.scalar.activation(lnr[:], rs[:], AF.Ln)
            nc.vector.tensor_scalar(u_m[:], lnr[:], -1.0, NORM, ALU.mult, ALU.add)
            # bin row u
            tb = sb.tile([1, 9], F32, tag="tb")
            nc.vector.tensor_tensor(tb[:], binr[:], v_r[:], ALU.add)
            tbe = sb.tile([1, 9], F32, tag="tbe")
            nc.scalar.activation(tbe[:], tb[:], AF.Exp)
            sb1 = sb.tile([1, 1], F32, tag="sb1")
            nc.vector.tensor_reduce(sb1[:], tbe[:], mybir.AxisListType.X, ALU.add)
            lb1 = sb.tile([1, 1], F32, tag="lb1")
            nc.scalar.activation(lb1[:], sb1[:], AF.Ln)
            nc.vector.tensor_scalar(u_b[:], lb1[:], -1.0, LOGN0 + NORM, ALU.mult, ALU.add)
            # ---- v update: v = log_nu - LSE_over_rows(C + u)
            CU = sb.tile([128, 288], F32, tag="T")
            nc.vector.tensor_tensor(CU[:], C[:], fview(u_m[:], [[1, 32], [0, 9]]), ALU.add)
            CUe = sb.tile([128, 288], F32, tag="Te")
            nc.scalar.activation(CUe[:], CU[:], AF.Exp)
            ps_cs2 = ps.tile([128, 512], F32, tag="psB")
            K.mm(ps_cs2[0:1, 0:288], ones[:, 0:1], CUe[:])
            cs2 = sb.tile([1, 288], F32, tag="cs2")
            nc.vector.tensor_copy(cs2[:], ps_cs2[0:1, 0:288])
            s9 = sb.tile([1, 9], F32, tag="s9")
            nc.vector.tensor_reduce(s9[:], fview(cs2[:], [[1, 9], [9, 32]]), mybir.AxisListType.X, ALU.add)
            # add bin-row exp(binr + u_b)
            tb2 = sb.tile([1, 9], F32, tag="tb")
            nc.vector.tensor_scalar_add(tb2[:], binr[:], u_b[0:1, 0:1])
            tbe2 = sb.tile([1, 9], F32, tag="tbe2")
            nc.scalar.activation(tbe2[:], tb2[:], AF.Exp)
            s9f = sb.tile([1, 9], F32, tag="s9f")
            nc.vector.tensor_tensor(s9f[:], s9[:], tbe2[:], ALU.add)
            l9 = sb.tile([1, 9], F32, tag="l9")
            nc.scalar.activation(l9[:], s9f[:], AF.Ln)
            nc.vector.tensor_scalar(v_r[:], l9[:], -1.0, NORM, ALU.mult, ALU.add)
            # log_nu last entry is log(m)+norm: fix bin col
            nc.vector.tensor_scalar_add(v_r[0:1, 8:9], v_r[0:1, 8:9], LOGM)

        # final scores F = C + u + v - norm ; output row = F[row0, 0:8]
        ps_vb = ps.tile([128, 512], F32, tag="psA")
        K.mm(ps_vb[:, 0:9], ones[0:1, 0:128], v_r[:], tp=(0, 0))
        vb = sb.tile([128, 9], F32, tag="vb")
        nc.vector.tensor_copy(vb[:], ps_vb[:, 0:9])
        Ff = sb.tile([128, 288], F32, tag="Ff")
        nc.vector.tensor_tensor(Ff[:], C[:], fview(vb[:], [[0, 32], [1, 9]]), ALU.add)
        nc.vector.tensor_tensor(Ff[:], Ff[:], fview(u_m[:], [[1, 32], [0, 9]]), ALU.add)
        nc.vector.tensor_scalar_add(Ff[:], Ff[:], -NORM)
        # min/max over cols (exclude ik9==8)
        mn = sb.tile([128, 2], F32, tag="mn")
        nc.vector.tensor_reduce(mn[:, 0:1], fview(Ff[:], [[9, 32], [1, 8]]), mybir.AxisListType.XY, ALU.min)
        nc.vector.tensor_reduce(mn[:, 1:2], fview(Ff[:], [[9, 32], [1, 8]]), mybir.AxisListType.XY, ALU.max)
        ps_tm = ps.tile([128, 512], F32, tag="psB")
        nc.tensor.transpose(ps_tm[0:2, 0:128], mn[:], ident[:])
        mnT = sb.tile([2, 128], F32, tag="mnT")
        nc.vector.tensor_copy(mnT[:], ps_tm[0:2, 0:128])
        mm2 = sb.tile([1, 1], F32, tag="mm2")
        nc.vector.tensor_reduce(mm2[:], mnT[0:1, :], mybir.AxisListType.X, ALU.min)
        mxrow = sb.tile([1, 128], F32, tag="mxrow")
        nc.sync.dma_start(mxrow[:], mnT[1:2, :])
        mx0 = sb.tile([1, 1], F32, tag="mx0")
        nc.vector.tensor_reduce(mx0[:], mxrow[:], mybir.AxisListType.X, ALU.max)
        smin = sb.tile([1, 1], F32, tag="smin")
        nc.vector.tensor_tensor(smin[:], mm2[:], mx0[:], ALU.subtract)
        nc.vector.tensor_scalar_add(smin[:], smin[:], -40.0)
        # row0 = C[0, 0:8] + u[0,0] + v[0:8] - norm
        row0 = sb.tile([1, 8], F32, tag="row0")
        nc.vector.tensor_scalar_add(row0[:], C[0:1, 0:8], u_m[0:1, 0:1])
        nc.vector.tensor_tensor(row0[:], row0[:], v_r[0:1, 0:8], ALU.add)
        nc.vector.tensor_scalar_add(row0[:], row0[:], -NORM)
        # out = (row0 + smin*maskf) * 15
        tmp8 = sb.tile([1, 8], F32, tag="tmp8")
        nc.vector.tensor_scalar(tmp8[:], maskf[:], smin[0:1, 0:1], None, ALU.mult)
        outr = sb.tile([1, 8], F32, tag="outr")
        nc.vector.tensor_tensor(outr[:], row0[:], tmp8[:], ALU.add)
        nc.vector.tensor_scalar_mul(outr[:], outr[:], 15.0)
        nc.sync.dma_start(out_t[:], outr[:])

        if DBG:
            nc.sync.dma_start(dbg["dbg_desc0"][:], d0[:])
            nc.sync.dma_start(dbg["dbg_desc2"][:], d2[:])
            nc.sync.dma_start(dbg["dbg_desc3"][:], d3[:])
            nc.sync.dma_start(dbg["dbg_d1loc"][:], d1l[:])
            nc.sync.dma_start(dbg["dbg_z"][:], zt[:])
            nc.sync.dma_start(dbg["dbg_row0"][0:1, 0:8], row0[:])
            nc.sync.dma_start(dbg["dbg_row0"][0:1, 8:9], smin[:])
        for p in (K.dram, K.ps, K.pst, K.sb1, K.sb, K.w):
            p.release()
    return nc


_CACHE = {}


def _get_nc(DBG=False):
    key = DBG
    if key not in _CACHE:
        nc = bacc.Bacc("TRN2", target_bir_lowering=False, debug=False, num_devices=NCORES)
        build(nc, DBG)
        nc.compile()
        _CACHE[key] = nc
    return _CACHE[key]


def make_in_maps(inputs):
    params = inputs["params"]
    flat = pack_params(params)
    d0 = _np(inputs["desc0"])[0]
    d1 = _np(inputs["desc1"])[0]
    d2 = _np(inputs["desc2"])[0]
    d3 = _np(inputs["desc3"])[0]
    dist = _np(inputs["dist"])[0]  # (128, 8, 4096)
    maskf = (~np.asarray(inputs["mask"])).astype(np.float32).reshape(1, N0)
    av = float(np.asarray(params["bin_score"]).reshape(()))
    alpha_cols = np.full((D, 32), av, np.float32)
    alpha_row9 = np.full((1, 9), av, np.float32)
    ident = np.eye(D, dtype=np.float32)

    common = dict(desc0=d0, desc1=d1, desc2=d2, desc3=d3, maskf=maskf,
                  alpha_cols=alpha_cols, alpha_row9=alpha_row9,
                  ident=ident, **flat)
    in_maps = []
    for s in range(NCORES):
        m = dict(common)
        m["desc1_loc"] = np.ascontiguousarray(d1[:, NS * s : NS * s + NS])
        m["dist0_loc"] = np.ascontiguousarray(
            dist[:, :, NS * s : NS * s + NS].reshape(D, 8 * NS))
        # dist1 cols for d_lo = s: dist1_loc[ch=(i0,d_hi), i1] = dist[d_hi*8+s, i0, i1]
        m["dist1_loc"] = np.ascontiguousarray(
            dist[s::8, :, :].transpose(1, 0, 2).reshape(D, N1))
        in_maps.append(m)
    return in_maps


def kernel(desc0, desc1, desc2, desc3, dist, params, mask, _dbg=False, _trace=False):
    in_maps = make_in_maps(dict(desc0=desc0, desc1=desc1, desc2=desc2, desc3=desc3,
                                dist=dist, params=params, mask=mask))
    nc = _get_nc(_dbg)
    res = bass_utils.run_bass_kernel_spmd(nc, in_maps, core_ids=list(range(NCORES)),
                                          trace=_trace)
    out = res.results[0]["out"].reshape(1, N0).astype(np.float32)
    if _dbg or _trace:
        return out, res
    return out
